# revision 43
# baseline (speedup 1.0000x reference)
"""MoE router kernel for Trainium2 (8 NeuronCores, SPMD data-parallel).

Computes, for x [B,S,H] and gate_w [E,H]:
    logits = x @ gate_w.T           # [B,S,E]
    p = softmax(logits, -1)
    w, i = top_k(p, 2); w = w / w.sum(-1, keepdims=True)

Math used on-device: renormalized top-2 softmax weights collapse to
    w1 = sigmoid(l1 - l2), w2 = sigmoid(l2 - l1)
where l1 >= l2 are the top-2 logits, so the full softmax is never needed.

Sharding: tokens (B*S = 16384) split evenly across 8 cores; gate weights
replicated. Per core: 2048 tokens x 4096 hidden.

Per-core pipeline (groups of 512 tokens = 4 x-tiles of [128, 4096]):
  DMA x tile [128, 4096] fp32 -> PE fp32 transposes of 128x128 chunks
  into PSUM banks [128h, 512t] -> DVE/ACT copy to SBUF -> fp32r GEMM
  (w chunk [128h, 64e] stationary, xT [128h, 512t] moving, 1 cyc/row)
  accumulating logitsT [64e, 512t] in PSUM -> copy to SBUF -> 4 PE
  transposes back to [128t, 64e] -> DVE max/max_index (top-8 sorted) ->
  ACT sigmoid -> DMA out.

fp32r (reduced-precision fp32 matmul, ~12.7 effective mantissa bits
measured on HW) is used ONLY for the final contraction against the tiny
gate weights; x itself is transposed at full fp32. Measured index
mismatch stays well inside the 2e-2 gate. Set GEMM_MODE="fp16x3" for a
bit-near-fp32 3-pass split-fp16 GEMM instead (slower, ~20 mantissa
bits).
"""

import sys

sys.path.insert(0, "/opt/trn_rl_repo")

import numpy as np

import concourse.bass as bass
import concourse.mybir as mybir
import concourse.tile as tile
from concourse.bass_utils import run_bass_kernel_spmd
import orjson
import concourse.bass_utils as _bu
import concourse.bass2jax as _b2j

_orig_compile_bir = _bu.compile_bir_kernel


def _legalize_waits(bir_json: bytes) -> bytes:
    """This walrus build allows only ONE sync-wait per compute
    instruction; move excess waits onto a Drain inserted just before
    (Drain accepts one wait each)."""
    m = orjson.loads(bir_json)
    changed = False
    for fn in m["functions"]:
        for blk in fn["blocks"]:
            out = []
            for inst in blk["instructions"]:
                si = inst.get("sync_info")
                w = (si or {}).get("on_wait") or []
                if len(w) > 1:
                    for k, wk in enumerate(w[:-1]):
                        out.append({
                            "debug": inst.get("debug", 0),
                            "engine": inst["engine"],
                            "ins": [], "outs": [],
                            "name": inst["name"] + f"-lw{k}",
                            "opcode": "Drain",
                            "sync_info": {"on_update": [], "on_wait": [wk]},
                        })
                    si["on_wait"] = w[-1:]
                    changed = True
                out.append(inst)
            blk["instructions"] = out
    return orjson.dumps(m) if changed else bir_json


def _compile_bir_legalized(bir_json, tmpdir, neff_name="file.neff"):
    return _orig_compile_bir(_legalize_waits(bir_json), tmpdir, neff_name)


_bu.compile_bir_kernel = _compile_bir_legalized
_b2j.compile_bir_kernel = _compile_bir_legalized

F32 = mybir.dt.float32
F32R = mybir.dt.float32r
F16 = mybir.dt.float16
U32 = mybir.dt.uint32

B, S, H, E = 4, 4096, 4096, 64
N_CORES = 8
P = 128                      # partitions / tile height
TOK_TOTAL = B * S            # 16384
TOK = TOK_TOTAL // N_CORES   # 2048 tokens per core
NCH = H // P                 # 32 contraction chunks of 128
GTOK = 512                   # tokens per GEMM group (PSUM bank = 512 fp32)
NTPG = GTOK // P             # 4 x-tiles per group
NGRP = TOK // GTOK           # 4 groups per core
GEMM_LAG = 4                 # chunks the GEMM trails the transposes by

GEMM_MODE = "f32r"           # "f32r" | "fp16x3"


def build_nc(tok: int = TOK):
    """Build the per-core Bass program (SPMD: same program, 8 cores)."""
    nc = bass.Bass()

    # In f32r mode the whole x path is *declared* float32r so the BIR
    # verifier accepts the f32r GEMM; the bits are plain fp32 end-to-end
    # (DMA and transpose-mode PE moves don't round).
    XDT = F32R if GEMM_MODE == "f32r" else F32

    ntiles = tok // P
    x_ext = nc.declare_dram_parameter("x", [tok, H], XDT, isOutput=False)
    id_ext = nc.declare_dram_parameter("ident", [P, P], XDT, isOutput=False)
    id2_ext = nc.declare_dram_parameter("ident2", [P, P], F32, isOutput=False)
    # outputs are partition-major [p, tile, 2] so the final flush is a
    # single DMA with 128B-contiguous runs per partition; the host
    # transposes back to [tok, 2]
    ow_ext = nc.declare_dram_parameter("out_w", [P, ntiles, 2], F32,
                                       isOutput=True)
    oi_ext = nc.declare_dram_parameter("out_i", [P, ntiles, 2], U32,
                                       isOutput=True)
    if GEMM_MODE == "f32r":
        wt_ext = nc.declare_dram_parameter("wt", [P, NCH, E], F32R,
                                           isOutput=False)
    else:
        whi_ext = nc.declare_dram_parameter("whi", [P, NCH, E], F16,
                                            isOutput=False)
        wlo_ext = nc.declare_dram_parameter("wlo", [P, NCH, E], F16,
                                            isOutput=False)

    with tile.TileContext(nc) as tc:
        with (
            tc.tile_pool(name="consts", bufs=1) as consts,
            tc.tile_pool(name="xin", bufs=8) as xpool,
            tc.tile_pool(name="xcol", bufs=8) as xcpool,
            tc.tile_pool(name="xt", bufs=8) as xtpool,
            tc.tile_pool(name="small", bufs=4) as small,
            tc.tile_pool(name="outp", bufs=8) as outp,
        ):
            # Consts first (small, ~0.4us of stream), then x tiles follow
            # on the same sync queue so tile 0 lands right behind them.
            if GEMM_MODE == "f32r":
                wt_sb = consts.tile([P, NCH, E], F32R)
                nc.sync.dma_start(wt_sb[:], wt_ext[:])
            else:
                whi_sb = consts.tile([P, NCH, E], F16)
                nc.sync.dma_start(whi_sb[:], whi_ext[:])
                wlo_sb = consts.tile([P, NCH, E], F16)
                nc.sync.dma_start(wlo_sb[:], wlo_ext[:])
            id_sb = consts.tile([P, P], XDT)
            nc.sync.dma_start(id_sb[:], id_ext[:])
            id2_sb = consts.tile([P, P], F32)
            nc.sync.dma_start(id2_sb[:], id2_ext[:])

            # Primers: each engine's first instruction carries a preamble
            # self-guard wait and fused LDWEIGHTS can hold just one wait,
            # so give every engine a first op with no other dependency
            # (const APs are pre-TileContext, untracked), and absorb each
            # const-DMA sem into a throwaway PE op. The scr pool is opened
            # and closed before the main PSUM pools so its banks are free
            # for the pipeline.
            prim = consts.tile([P, 2], F32)
            nc.vector.memset(prim[:, 0:1], 0.0)
            nc.scalar.copy(prim[:, 1:2], nc.const_aps.tensor(1.0, (P, 1)))
            with tc.tile_pool(name="scr", bufs=1, space="PSUM") as scr_pool:
                scr = scr_pool.tile([P, P], XDT)
                nc.tensor.matmul(scr[:], id_sb[:], id_sb[:],
                                 is_transpose=True, start=True, stop=True)
                scr2 = scr_pool.tile([P, P], F32)
                nc.tensor.matmul(scr2[:], id2_sb[:], id2_sb[:],
                                 is_transpose=True, start=True, stop=True)
                if GEMM_MODE == "f32r":
                    nc.tensor.matmul(scr2[0:E, :], wt_sb[:, 0, :],
                                     id_sb[:],
                                     start=True, stop=True)
                else:
                    nc.tensor.matmul(scr[0:E, 0:E], whi_sb[:, 0, :],
                                     whi_sb[:, 0, :], start=True, stop=True)
                    nc.tensor.matmul(scr[0:E, 0:E], wlo_sb[:, 0, :],
                                     wlo_sb[:, 0, :], start=False, stop=True)

            def emit_gemm(lg_ps, xt_tiles, c):
                """logitsT[e, tok] += wT[c].T-free GEMM over chunk c."""
                if GEMM_MODE == "f32r":
                    nc.tensor.matmul(
                        lg_ps[:], wt_sb[:, c, :], xt_tiles[c][:],
                        start=(c == 0), stop=(c == NCH - 1),
                    )
                else:
                    xr = xt_tiles[c]
                    nc.tensor.matmul(lg_ps[:], whi_sb[:, c, :], xr["hi"][:],
                                     start=(c == 0), stop=False)
                    nc.tensor.matmul(lg_ps[:], wlo_sb[:, c, :], xr["hi"][:],
                                     start=False, stop=False)
                    nc.tensor.matmul(lg_ps[:], whi_sb[:, c, :], xr["lo"][:],
                                     start=False, stop=(c == NCH - 1))

            with (
                tc.tile_pool(name="ps_t", bufs=5, space="PSUM") as ps_t,
                tc.tile_pool(name="ps_l", bufs=2, space="PSUM") as ps_l,
                tc.tile_pool(name="ps_b", bufs=1, space="PSUM") as ps_b,
            ):
                copy_flip = 0

                def chunk_ap(xt, c):
                    """Chunk-c column slice; xt is a tile or a list of 4
                    column tiles (fine-grained DMA dependencies)."""
                    if isinstance(xt, list):
                        return xt[c // 8][:, (c % 8) * P:(c % 8 + 1) * P]
                    return xt[:, c * P:(c + 1) * P]

                def do_transpose(x_tiles, xt_tiles, c):
                    """Transpose chunk c of the group's tiles -> PSUM -> SBUF."""
                    nonlocal copy_flip
                    ntl = len(x_tiles)
                    gt = ntl * P
                    xT_ps = ps_t.tile([P, ntl, P], XDT)
                    for ti in range(ntl):
                        nc.tensor.matmul(
                            xT_ps[:, ti, :],
                            chunk_ap(x_tiles[ti], c),
                            id_sb[:],
                            is_transpose=True,
                            start=(ti == 0),
                            stop=(ti == ntl - 1),
                        )
                    if GEMM_MODE == "f32r":
                        xT_sb = xtpool.tile([P, gt], XDT)
                        if copy_flip % 2 == 0:
                            nc.vector.tensor_copy(xT_sb[:], xT_ps[:])
                        else:
                            nc.scalar.copy(xT_sb[:], xT_ps[:])
                        copy_flip += 1
                        xt_tiles[c] = xT_sb
                    else:
                        hi = xtpool.tile([P, gt], F16)
                        lo = xtpool.tile([P, gt], F16)
                        if copy_flip % 2 == 0:
                            nc.vector.tensor_copy(hi[:], xT_ps[:])
                            nc.scalar.tensor_tensor(
                                lo[:], xT_ps[:], hi[:],
                                mybir.AluOpType.subtract)
                        else:
                            nc.scalar.copy(hi[:], xT_ps[:])
                            nc.vector.tensor_tensor(
                                lo[:], xT_ps[:], hi[:],
                                mybir.AluOpType.subtract)
                        copy_flip += 1
                        xt_tiles[c] = {"hi": hi, "lo": lo}

                ow_all = consts.tile([P, ntiles, 2], F32)
                oi_all = consts.tile([P, ntiles, 2], U32)

                def emit_output(tok0, ntl, lg_ps):
                    """Transpose logitsT back to [tok, e]; top-2 into the
                    output accumulators (flushed once at kernel end)."""
                    gt = ntl * P
                    lgT_sb = small.tile([E, gt], F32)
                    nc.vector.tensor_copy(lgT_sb[:], lg_ps[:])
                    lg_bk = ps_b.tile([P, ntl, E], F32)
                    for j in range(ntl):
                        nc.tensor.matmul(
                            lg_bk[:, j, :],
                            lgT_sb[:, j * P:(j + 1) * P],
                            id2_sb[0:E, 0:E],
                            is_transpose=True,
                            start=(j == 0),
                            stop=(j == ntl - 1),
                        )
                    lg_sb = small.tile([P, ntl, E], F32)
                    nc.vector.tensor_copy(lg_sb[:], lg_bk[:])

                    for j in range(ntl):
                        gj = tok0 // P + j
                        mx = outp.tile([P, 8], F32)
                        nc.vector.max(mx[:], lg_sb[:, j, :])
                        ix = outp.tile([P, 8], U32)
                        nc.vector.max_index(ix[:], mx[:], lg_sb[:, j, :])
                        # top-2 logits; sigmoid renorm runs on the host
                        nc.vector.tensor_copy(ow_all[:, gj, :], mx[:, 0:2])
                        nc.vector.tensor_copy(oi_all[:, gj, :], ix[:, 0:2])

                # Ramp-up / ramp-down groups: single tiles at both ends
                # (fp32r GEMM drops to 4 cyc/row below 256 moving, but this
                # fills PE time that would otherwise idle waiting for DMAs
                # and shortens the tail); the first and last tiles are
                # loaded as 4 separate column tiles so transposes start
                # after 1/4 of the tile lands.
                groups = [(0, 1), (128, 1), (256, 2), (512, 4), (1024, 4),
                          (1536, 2), (1792, 1), (1920, 1)]
                assert sum(n for _, n in groups) * P == TOK
                split_first = 0
                split_last = len(groups) - 1

                pending_out = None
                for gi, (tok0, ntl) in enumerate(groups):
                    x_tiles = []
                    for ti in range(ntl):
                        t0 = tok0 + ti * P
                        if gi in (split_first, split_last):
                            cols = []
                            for k in range(4):
                                xc = xcpool.tile([P, H // 4], XDT)
                                nc.sync.dma_start(
                                    xc[:],
                                    x_ext[t0:t0 + P,
                                          k * (H // 4):(k + 1) * (H // 4)])
                                cols.append(xc)
                            x_tiles.append(cols)
                        else:
                            x_sb = xpool.tile([P, H], XDT)
                            nc.sync.dma_start(x_sb[:], x_ext[t0:t0 + P, :])
                            x_tiles.append(x_sb)

                    lg_ps = ps_l.tile([E, ntl * P], F32)
                    xt_tiles = {}

                    # software pipeline: transposes run GEMM_LAG chunks
                    # ahead; the previous group's output stage is emitted
                    # into this group's transpose stream.
                    for c in range(NCH):
                        do_transpose(x_tiles, xt_tiles, c)
                        if c == 3 and pending_out is not None:
                            emit_output(*pending_out)
                            pending_out = None
                        if c >= GEMM_LAG:
                            emit_gemm(lg_ps, xt_tiles, c - GEMM_LAG)
                            xt_tiles.pop(c - GEMM_LAG)
                    for c in range(NCH - GEMM_LAG, NCH):
                        emit_gemm(lg_ps, xt_tiles, c)
                        xt_tiles.pop(c)
                    pending_out = (tok0, ntl, lg_ps)

                emit_output(*pending_out)
                nc.sync.dma_start(ow_ext[:], ow_all[:])
                nc.sync.dma_start(oi_ext[:], oi_all[:])

    return nc


_NC_CACHE = {}


def _get_nc(tok: int):
    if tok not in _NC_CACHE:
        _NC_CACHE[tok] = build_nc(tok)
    return _NC_CACHE[tok]


def make_in_maps(x: np.ndarray, gate_w: np.ndarray):
    """Shard full inputs into per-core input maps."""
    xf = np.ascontiguousarray(x.reshape(TOK_TOTAL, H), dtype=np.float32)
    # wt[p, c, e] = gate_w[e, 128*c + p]
    wt = np.ascontiguousarray(
        gate_w.T.reshape(NCH, P, E).transpose(1, 0, 2), dtype=np.float32
    )
    ident = np.eye(P, dtype=np.float32)
    common = {"ident": ident, "ident2": ident}
    if GEMM_MODE == "f32r":
        common["wt"] = wt
    else:
        whi = wt.astype(np.float16)
        wlo = (wt - whi.astype(np.float32)).astype(np.float16)
        common["whi"] = whi
        common["wlo"] = wlo
    return [
        {"x": np.ascontiguousarray(xf[i * TOK:(i + 1) * TOK]), **common}
        for i in range(N_CORES)
    ]


def kernel(x, gate_w, _trace: bool = False):
    x = np.asarray(x, dtype=np.float32)
    gate_w = np.asarray(gate_w, dtype=np.float32)
    nc = _get_nc(TOK)
    in_maps = make_in_maps(x, gate_w)
    res = run_bass_kernel_spmd(
        nc, in_maps, core_ids=list(range(N_CORES)), trace=_trace
    )
    # device output is partition-major [p, tile, 2]; token = tile*128 + p
    lg12 = np.concatenate(
        [res.results[i]["out_w"].transpose(1, 0, 2).reshape(TOK, 2)
         for i in range(N_CORES)])
    out_i = np.concatenate(
        [res.results[i]["out_i"].transpose(1, 0, 2).reshape(TOK, 2)
         for i in range(N_CORES)])
    # device ships the top-2 logits (l1, l2); renormalized top-2 softmax
    # weights collapse to sigmoids of the logit difference
    d12 = lg12[:, 0] - lg12[:, 1]
    w1 = 1.0 / (1.0 + np.exp(-d12))
    w2 = 1.0 / (1.0 + np.exp(d12))
    topk_weights = np.stack([w1, w2], axis=-1).astype(np.float32).reshape(B, S, 2)
    topk_indices = out_i.astype(np.int32).reshape(B, S, 2)
    if _trace:
        kernel._last_result = res
    return topk_weights, topk_indices
